# revision 28
# baseline (speedup 1.0000x reference)
"""Trainium2 Bass kernel for nn_ActionModuleTorch (sparse action attention).

Sharding: data-parallel over spatial positions S=880 (padded to 896 -> 112
positions per core on 8 cores), no collectives. Weights replicated, host-cast
to bf16. Activations are feature-major ([C on partitions, (s,t) tokens on the
free dim, frame t minor]) so all six big GEMMs chain with natural-layout bf16
weights as the stationary matmul operand.

Cross-partition reductions (layernorm / rmsnorm / softmax denominators) use
ones/selector matmuls on the PE; per-token statistics are broadcast back
across partitions with outer-product matmuls. Mouse attention batches 8
positions per matmul with a block-diagonal mask (off-diagonal cross-position
scores are killed by exp(-30000) -> 0, which also makes the softmax
denominator a single ones-reduction). V is produced row-major by swapping the
matmul operand roles so attention output lands feature-major for the
projection GEMMs.
"""

import sys

import numpy as np

sys.path.insert(0, "/opt/trn_rl_repo")

import ml_dtypes  # noqa: E402

import concourse.bass as bass  # noqa: E402
import concourse.mybir as mybir  # noqa: E402
from concourse import bacc  # noqa: E402
from concourse import tile_utils  # noqa: E402
from concourse.bass_utils import run_bass_kernel_spmd  # noqa: E402
from concourse.tile import TileContext  # noqa: E402

tile_utils.max_sbuf_usage = 219 * 1024  # stale 192KiB default; 224KiB phys

BF = mybir.dt.bfloat16
F32 = mybir.dt.float32
bf16 = ml_dtypes.bfloat16

T = 16
S = 880
SP = 896
NCORES = 8
NSC = SP // NCORES        # 112 positions/core
NT = NSC * T              # 1792 tokens/core
C = 1536
H, D, HD, MHD = 16, 64, 1024, 1024
THETA, T_DIM = 256.0, 8
RATIO, WIN, LOCAL = 4, 3, 6
FW = RATIO * WIN          # 12

CT = 256                  # tokens per chunk (16 positions)
PC = CT // T              # 16 positions per chunk
NTC = NT // CT            # 7 chunks per core
GC = 2                    # 128-col attention groups per chunk

_CACHE = {}


# ---------------------------------------------------------------- host tables
def _rope_tables():
    freqs = 1.0 / (THETA ** (np.arange(0, T_DIM, 2, dtype=np.float64) / T_DIM))
    ang = np.arange(T, dtype=np.float64)[:, None] * freqs[None, :]
    cos = np.concatenate([np.repeat(np.cos(ang), 2, 1), np.ones((T, D - T_DIM))], 1)
    sin = np.concatenate([np.repeat(np.sin(ang), 2, 1), np.zeros((T, D - T_DIM))], 1)
    return cos.astype(np.float32), sin.astype(np.float32)  # [t, d]


def _perm_rot(w, scale):
    """lhsT [128,128] for rot = P @ x per 64-head-block, weight+scale folded."""
    P = np.zeros((D, D), np.float32)
    for i in range(T_DIM // 2):
        P[2 * i, 2 * i + 1] = -1.0
        P[2 * i + 1, 2 * i] = 1.0
    PwT = (P * (w[None, :] * scale)).T  # [d, j]
    out = np.zeros((128, 128), np.float32)
    out[:D, :D] = PwT
    out[D:, D:] = PwT
    return out


def _cos_table(cos, w, scale):
    m = (cos * (w[None, :] * scale)).T  # [d, t]
    return np.concatenate([m, m], axis=0)  # [128, 16]


def _consts(inp):
    cos, sin = _rope_tables()
    a = {}
    a["sint"] = np.concatenate([sin.T, sin.T], axis=0)
    a["cosm_q"] = _cos_table(cos, np.asarray(inp["qn_m"], np.float32), 0.125)
    a["cosm_k"] = _cos_table(cos, np.asarray(inp["kn_m"], np.float32), 1.0)
    a["cosk_q"] = _cos_table(cos, np.asarray(inp["qn_k"], np.float32), 0.125)
    a["cosk_k"] = _cos_table(cos, np.asarray(inp["kn_k"], np.float32), 1.0)
    a["pm_q"] = _perm_rot(np.asarray(inp["qn_m"], np.float32), 0.125)
    a["pm_k"] = _perm_rot(np.asarray(inp["kn_m"], np.float32), 1.0)
    a["pk_q"] = _perm_rot(np.asarray(inp["qn_k"], np.float32), 0.125)
    a["pk_k"] = _perm_rot(np.asarray(inp["kn_k"], np.float32), 1.0)

    i = np.arange(T)
    mask = (i[None, :] <= i[:, None]) & (i[:, None] - i[None, :] < LOCAL)
    mbT = np.where(mask.T, 0.0, -30000.0).astype(np.float32)  # [t', t]
    md = np.full((128, 128), -30000.0, np.float32)
    for g in range(8):
        md[g * T:(g + 1) * T, g * T:(g + 1) * T] = mbT
    a["maskd"] = md                       # mouse: block-diag over 8 positions
    a["maskk"] = np.tile(mbT, (8, 1))     # kb: 8 head-blocks stacked

    # sel16[p, m, c] = 1 iff c == 2m + p//64 : per-chunk head sumsq selector
    sel16 = np.zeros((128, 8, 16), np.float32)
    for m in range(8):
        sel16[:64, m, 2 * m] = 1.0
        sel16[64:, m, 2 * m + 1] = 1.0
    a["sel16"] = sel16
    # colsel[p, h, c] = 1 iff c == h : full-column-sum into row h
    colsel = np.zeros((128, 16, 16), np.float32)
    for h in range(16):
        colsel[:, h, h] = 1.0
    a["colsel"] = colsel
    a["colsel16"] = colsel[:16].copy()  # [16, 16, 16] for kb denominators
    a["ones128"] = np.ones((128, 1), np.float32)
    a["onesr"] = np.ones((1, 128), np.float32)
    repsel = np.zeros((16, 1024), np.float32)
    for cc in range(8):
        for hh in range(2):
            repsel[2 * cc + hh, cc * 128 + hh * 64: cc * 128 + (hh + 1) * 64] = 1.0
    a["repsel"] = repsel
    repsel4 = np.zeros((4, 256), np.float32)
    for cl in range(2):
        for hh in range(2):
            repsel4[2 * cl + hh, cl * 128 + hh * 64: cl * 128 + (hh + 1) * 64] = 1.0
    a["repsel4"] = repsel4
    return a


def _pack_w(w):
    """[K, M] -> [128, K//128, M] (partition-major for direct DMA)."""
    K, M = w.shape
    return np.ascontiguousarray(w.reshape(K // 128, 128, M).transpose(1, 0, 2))


def _pack_b(v):
    """[C] -> [128, C//128]."""
    return np.ascontiguousarray(v.reshape(-1, 128).T)


def _prep_host(inp):
    f = {k: np.asarray(v, np.float32) for k, v in inp.items()
         if k not in ("tt", "th", "tw")}
    sh = {}
    mm_w1 = f["mm_w1"]
    sh["w1"] = _pack_w(mm_w1[:C]).astype(bf16)
    wg = np.zeros((128, MHD), np.float32)
    wg[:FW * 2] = mm_w1[C:]
    sh["wg"] = wg.astype(bf16)
    sh["b1"] = _pack_b(f["mm_b1"])
    sh["w2"] = _pack_w(f["mm_w2"]).astype(bf16)
    sh["b2"] = _pack_b(f["mm_b2"])

    wqkv = f["mm_ln_g"][:, None] * f["t_qkv_w"]
    bqkv = f["mm_ln_b"] @ f["t_qkv_w"]
    sh["wq3"] = _pack_w(wqkv[:, :HD]).astype(bf16)
    sh["wk3"] = _pack_w(wqkv[:, HD:2 * HD]).astype(bf16)
    sh["wv3"] = _pack_w(wqkv[:, 2 * HD:]).astype(bf16)
    sh["bqkv"] = _pack_b(bqkv[:2 * HD])          # [128, 16] q|k biases
    sh["bqv"] = bqkv[2 * HD:].astype(bf16)[None, :]  # [1, 1024]
    sh["wpm"] = _pack_w(f["proj_mouse_w"]).astype(bf16)
    sh["wqk"] = _pack_w(f["mouse_q_w"]).astype(bf16)
    sh["wkv"] = _pack_w(f["kb_kv_w"]).astype(bf16)
    sh["wpk"] = _pack_w(f["proj_kb_w"]).astype(bf16)

    kbin = np.zeros((128, 72), np.float32)
    kbin[:6] = f["keyboard_condition"][0].T
    sh["kbin"] = kbin.astype(bf16)
    kw1 = np.zeros((128, 128), np.float32)
    kw1[:6] = f["kb_w1"]
    sh["kw1"] = kw1.astype(bf16)
    sh["kb1"] = f["kb_b1"].reshape(128, 1).copy()
    sh["kw2"] = f["kb_w2"].astype(bf16)
    sh["kb2"] = f["kb_b2"].reshape(128, 1).copy()

    mc = f["mouse_condition"][0]
    gm24 = np.stack([mc[4 * t:4 * t + FW].reshape(-1) for t in range(T)])
    gm24T = np.zeros((128, T), np.float32)
    gm24T[:FW * 2] = gm24.T
    sh["gm24"] = gm24T.astype(bf16)

    for k, v in _consts(inp).items():
        sh[k] = v.astype(np.float32) if k in ("maskd", "maskk") else v.astype(bf16)

    # x shard, feature-major, chunk-major DMA layout [128, 14, 12, 128]
    x = f["x"][0].reshape(T, S, C)
    xt = np.transpose(x, (2, 1, 0))                      # [C, s, t]
    xt_pad = np.zeros((C, SP, T), np.float32)
    xt_pad[:, :S] = xt
    per_core = []
    for c in range(NCORES):
        sl = xt_pad[:, c * NSC:(c + 1) * NSC, :].reshape(C, NT)
        # [C, NT] -> [128, NTC, 12, CT]
        arr = sl.reshape(12, 128, NTC, CT).transpose(1, 2, 0, 3)
        m = dict(sh)
        m["xtf"] = np.ascontiguousarray(arr)
        m["xtb"] = m["xtf"].astype(bf16)
        per_core.append(m)
    return per_core


# ------------------------------------------------------------------ builder
def _declare(nc, name, arr):
    dt = BF if arr.dtype == bf16 else F32
    return nc.declare_dram_parameter(name, list(arr.shape), dt, isOutput=False)


def build_graph(example_map):
    nc = bacc.Bacc()
    dram = {k: _declare(nc, k, v) for k, v in example_map.items()}
    out_ext = nc.declare_dram_parameter("out", [128, NTC, 12, CT], F32,
                                        isOutput=True)
    with TileContext(nc) as tc:
        _body(nc, tc, dram, out_ext)
    nc.finalize()
    return nc


def _body(nc, tc, dram, out_ext):
    TT = mybir.AluOpType
    AF = mybir.ActivationFunctionType
    MM = nc.tensor.matmul

    ctxs = []

    def pool(name, bufs, space="SBUF"):
        p = tc.tile_pool(name=name, bufs=bufs, space=space)
        ctxs.append(p)
        return p.__enter__()

    constp = pool("const", 1)
    bigp = pool("big", 2)       # whole-core activations, rotating slots
    wtsp = pool("wts", 2)       # streamed weights (24KB class)
    wts3p = pool("wts3", 1)     # v-weights slot during fused S3/S4 (16KB)
    strp = pool("stream", 2)    # xtb / hsb staging
    str1p = pool("stream1", 1)  # per-chunk single-buffer staging
    smallp = pool("small", 2)   # small per-(pair,chunk) temporaries
    ep = pool("eprobs", 16)     # mouse attention prob tiles
    ekbp = pool("ekb", 3)      # kb attention prob tiles
    ppG = pool("psG", 3, space="PSUM")
    pp1 = pool("ps1", 3, space="PSUM")
    ppA = pool("psacc", 2, space="PSUM")
    dramp = pool("drambounce", 1, space="DRAM")

    cnt = [0]

    def ps1(pcols=512, prow=128):
        cnt[0] += 1
        t = pp1.tile([128, 512], F32, tag="ps1", name=f"ps1_{cnt[0]}")
        return t[:prow, :pcols]

    def psG(pcols=512, prow=128):
        cnt[0] += 1
        t = ppG.tile([128, 512], F32, tag="psG", name=f"psG_{cnt[0]}")
        return t[:prow, :pcols]

    def psacc(pcols=512, prow=128):
        cnt[0] += 1
        t = ppA.tile([128, 512], F32, tag="psacc", name=f"psacc_{cnt[0]}")
        return t[:prow, :pcols]

    def dma(dst, src):
        nc.sync.dma_start(out=dst, in_=src)

    cst = {}
    for nm in ["cosm_q", "cosm_k", "cosk_q", "cosk_k", "sint", "pm_q", "pm_k",
               "pk_q", "pk_k", "maskd", "maskk", "sel16", "colsel", "colsel16",
               "ones128", "onesr", "repsel", "gm24", "wg", "kbin", "kw1", "kw2",
               "bqv", "b1", "b2", "bqkv", "kb1", "kb2", "repsel4"]:
        h = dram[nm]
        t = constp.tile(list(h.shape), h.dtype, tag=f"c_{nm}")
        dma(t[:], h.ap())
        cst[nm] = t

    def load_w(name, kchunks, mcols, pl=wtsp, tag="wts"):
        t = pl.tile([128, kchunks, mcols], BF, tag=tag)
        dma(t[:], dram[name].ap())
        return t

    # ---- G = wg^T @ gm24 + b1 : per-frame GEMM1 bias [1024, 16] ----
    g_sb = constp.tile([128, 8, T], F32, tag="g_sb")
    for m in range(8):
        ps = ps1(T)
        MM(ps, cst["wg"][:, m * 128:(m + 1) * 128], cst["gm24"][:],
           start=True, stop=True)
        nc.vector.tensor_tensor(g_sb[:, m], ps,
                                cst["b1"][:, m:m + 1].to_broadcast([128, T]),
                                TT.add)

    # ---- keyboard K/V (tiny, once per core) ----
    ps = ps1(72)
    MM(ps, cst["kw1"][:], cst["kbin"][:], start=True, stop=True)
    kc1 = smallp.tile([128, 72], F32, tag="kc1")
    nc.vector.tensor_tensor(kc1[:], ps, cst["kb1"][:].to_broadcast([128, 72]),
                            TT.add)
    kc1b = smallp.tile([128, 72], BF, tag="kc1b")
    nc.scalar.activation(kc1b[:], kc1[:], AF.Silu)
    ps = ps1(72)
    MM(ps, cst["kw2"][:], kc1b[:], start=True, stop=True)
    kc2 = constp.tile([128, 72], BF, tag="kc2")
    nc.vector.tensor_tensor(kc2[:], ps, cst["kb2"][:].to_broadcast([128, 72]),
                            TT.add)
    kc2v = kc2[:].rearrange("p (t r) -> p r t", r=4)  # stride-4 window view

    def gk_rhs(j):
        return kc2v[:, j % 4, j // 4: j // 4 + T]  # [128, 16] = gk^T chunk j

    wkv_k = wtsp.tile([128, 12, HD], BF, tag="wts", name="wkv_k")
    dma(wkv_k[:], dram["wkv"].ap()[:, :, :HD])
    kkT = constp.tile([128, 8, T], BF, tag="kkT")     # roped keyboard K^T
    kk_raw = constp.tile([128, 8, T], BF, tag="kk_raw")
    ss_k = psacc(T, T)
    for m in range(8):
        ps = ps1(T)
        for j in range(FW):
            MM(ps, wkv_k[:, j, m * 128:(m + 1) * 128], gk_rhs(j),
               start=(j == 0), stop=(j == FW - 1))
        nc.scalar.copy(kk_raw[:, m], ps)
        sq = smallp.tile([128, T], BF, tag="sq_kk")
        nc.scalar.activation(sq[:], kk_raw[:, m], AF.Square)
        MM(ss_k, cst["sel16"][:, m], sq[:], start=(m == 0), stop=(m == 7))
    rstd_k = smallp.tile([T, T], BF, tag="rstd_k")
    tmp = smallp.tile([T, T], F32, tag="t_rstdk")
    nc.vector.tensor_scalar(tmp[:], ss_k, 1.0 / 64, 1e-6, TT.mult, TT.add)
    nc.vector.reciprocal(tmp[:], tmp[:])
    nc.scalar.activation(rstd_k[:], tmp[:], AF.Sqrt)
    for m in range(8):
        rep = ps1(T)
        MM(rep, cst["repsel"][:, m * 128:(m + 1) * 128], rstd_k[:],
           start=True, stop=True)
        rot = ps1(T)
        MM(rot, cst["pk_k"][:], kk_raw[:, m], start=True, stop=True)
        t1 = smallp.tile([128, T], F32, tag="t1_kk")
        nc.vector.tensor_tensor(t1[:], kk_raw[:, m], cst["cosk_k"][:], TT.mult)
        t2 = smallp.tile([128, T], F32, tag="t2_kk")
        nc.vector.tensor_tensor(t2[:], rot, cst["sint"][:], TT.mult)
        nc.vector.tensor_tensor(t1[:], t1[:], t2[:], TT.add)
        nc.vector.tensor_tensor(kkT[:, m], t1[:], rep, TT.mult)
    wkv_v = wtsp.tile([128, 12, HD], BF, tag="wts", name="wkv_v")
    dma(wkv_v[:], dram["wkv"].ap()[:, :, HD:])
    vk = constp.tile([T, HD], BF, tag="vk")  # keyboard V rows [16, 1024]
    for nn in range(2):
        psv = psacc(512, T)
        for j in range(FW):
            MM(psv, gk_rhs(j), wkv_v[:, j, nn * 512:(nn + 1) * 512],
               start=(j == 0), stop=(j == FW - 1))
        nc.scalar.copy(vk[:, nn * 512:(nn + 1) * 512], psv)

    # ================= S1: GEMM1 x(+windowed mouse) -> h1 =================
    w1 = load_w("w1", 12, MHD)
    h1 = bigp.tile([128, 8, NT], BF, tag="big")
    for ti in range(NTC):
        xb = strp.tile([128, 12, CT], BF, tag="stream")
        dma(xb[:], dram["xtb"].ap()[:, ti])
        for mp in range(4):           # pairs of m-chunks share one psum bank
            ps = psG(512)
            for mh in range(2):
                m = 2 * mp + mh
                half = ps[:, mh * CT:(mh + 1) * CT]
                for k in range(12):
                    MM(half, w1[:, k, m * 128:(m + 1) * 128], xb[:, k],
                       start=(k == 0), stop=(k == 11))
                h3v = half.rearrange("p (s t) -> p s t", t=T)
                nc.vector.tensor_tensor(
                    h3v, h3v, g_sb[:, m, None, :].to_broadcast([128, PC, T]),
                    TT.add)
            nc.scalar.activation(
                h1[:, 2 * mp:2 * mp + 2, ti * CT:(ti + 1) * CT], ps,
                AF.Gelu_apprx_tanh)

    # ================= S2: GEMM2 + LayerNorm -> h3 =================
    w2 = load_w("w2", 8, MHD)
    h3 = bigp.tile([128, 8, NT], BF, tag="big")
    for ti in range(NTC):
        h2b = str1p.tile([128, 8, CT], BF, tag="h2b")
        st_mu = psacc(CT, 1)
        st_sq = psacc(CT, 1)
        for mp in range(4):
            ps = psG(512)
            for mh in range(2):
                m = 2 * mp + mh
                half = ps[:, mh * CT:(mh + 1) * CT]
                for k in range(8):
                    MM(half, w2[:, k, m * 128:(m + 1) * 128],
                       h1[:, k, ti * CT:(ti + 1) * CT],
                       start=(k == 0), stop=(k == 7))
                nc.scalar.activation(h2b[:, m], half, AF.Identity,
                                     bias=cst["b2"][:, m:m + 1])
        for m in range(8):
            sq = smallp.tile([128, CT], BF, tag="sq2", bufs=1)
            nc.scalar.activation(sq[:], h2b[:, m], AF.Square)
            MM(st_mu, cst["ones128"][:], h2b[:, m],
               start=(m == 0), stop=(m == 7))
            MM(st_sq, cst["ones128"][:], sq[:],
               start=(m == 0), stop=(m == 7))
        mu = smallp.tile([1, CT], F32, tag="mu")
        nc.vector.tensor_scalar(mu[:], st_mu, 1.0 / MHD, 0.0, TT.mult, TT.add)
        mu2 = smallp.tile([1, CT], F32, tag="mu2")
        nc.vector.tensor_tensor(mu2[:], mu[:], mu[:], TT.mult)
        var = smallp.tile([1, CT], F32, tag="ln_var")
        nc.vector.tensor_scalar(var[:], st_sq, 1.0 / MHD, 1e-5,
                                TT.mult, TT.add)
        nc.vector.tensor_tensor(var[:], var[:], mu2[:], TT.subtract)
        nc.vector.reciprocal(var[:], var[:])
        rstd = smallp.tile([1, CT], BF, tag="rstd2")
        nc.scalar.activation(rstd[:], var[:], AF.Sqrt)
        mub = smallp.tile([1, CT], BF, tag="mub")
        nc.vector.tensor_copy(mub[:], mu[:])
        rep_mu = ps1(CT)
        MM(rep_mu, cst["onesr"][:], mub[:], start=True, stop=True)
        rep_rs = ps1(CT)
        MM(rep_rs, cst["onesr"][:], rstd[:], start=True, stop=True)
        rs_sb = smallp.tile([128, CT], BF, tag="rs_sb")
        nc.scalar.copy(rs_sb[:], rep_rs)
        for mp in range(4):
            sl = slice(2 * mp, 2 * mp + 2)
            tmp = smallp.tile([128, 2, CT], BF, tag="ln_t", bufs=1)
            nc.vector.tensor_tensor(
                tmp[:], h2b[:, sl],
                rep_mu[:, None, :].to_broadcast([128, 2, CT]), TT.subtract)
            nc.vector.tensor_tensor(
                h3[:, sl, ti * CT:(ti + 1) * CT], tmp[:],
                rs_sb[:, None, :].to_broadcast([128, 2, CT]), TT.mult)

    # ---- shared rms+rope epilogue, pair-packed ----
    def rope_apply(raw_sb, ss, out_ap, cosname, pname):
        rstd = smallp.tile([T, CT], BF, tag="rstd3")
        tmp = smallp.tile([T, CT], F32, tag="t_rstd3", bufs=1)
        nc.vector.tensor_scalar(tmp[:], ss, 1.0 / 64, 1e-6, TT.mult, TT.add)
        nc.vector.reciprocal(tmp[:], tmp[:])
        nc.scalar.activation(rstd[:], tmp[:], AF.Sqrt)
        for mp in range(4):
            rep = ps1(512)
            rot = ps1(512)
            for mh in range(2):
                m = 2 * mp + mh
                MM(rep[:, mh * CT:(mh + 1) * CT],
                   cst["repsel"][:, m * 128:(m + 1) * 128], rstd[:],
                   start=True, stop=True)
                MM(rot[:, mh * CT:(mh + 1) * CT], cst[pname][:], raw_sb[:, m],
                   start=True, stop=True)
            t1 = smallp.tile([128, 512], BF, tag="t1_r", bufs=1)
            nc.vector.tensor_tensor(
                t1[:].rearrange("p (s t) -> p s t", t=T),
                raw_sb[:, 2 * mp:2 * mp + 2].rearrange("p a b -> p (a b)")
                .rearrange("p (s t) -> p s t", t=T),
                cst[cosname][:, None, :].to_broadcast([128, 2 * PC, T]),
                TT.mult)
            t2 = smallp.tile([128, 512], BF, tag="t2_r", bufs=1)
            nc.vector.tensor_tensor(
                t2[:].rearrange("p (s t) -> p s t", t=T),
                rot.rearrange("p (s t) -> p s t", t=T),
                cst["sint"][:, None, :].to_broadcast([128, 2 * PC, T]),
                TT.mult)
            nc.vector.tensor_tensor(t1[:], t1[:], t2[:], TT.add)
            nc.vector.tensor_tensor(out_ap(mp),
                                    t1[:].rearrange("p (a b) -> p a b", b=CT),
                                    rep.rearrange("p (a b) -> p a b", b=CT),
                                    TT.mult)

    # ============ S3+S4 fused: qkv GEMM + rms/rope + mouse attention ============
    wq3 = load_w("wq3", 8, HD)
    wk3 = load_w("wk3", 8, HD)
    wv3 = load_w("wv3", 8, HD, pl=wts3p, tag="wts3")
    attnT = bigp.tile([128, 8, NT], BF, tag="big")
    for ti in range(NTC):
        cols = slice(ti * CT, (ti + 1) * CT)
        qr = str1p.tile([128, 8, CT], BF, tag="qraw", bufs=2)
        kr = str1p.tile([128, 8, CT], BF, tag="kraw", bufs=2)
        ss_q = psacc(CT, T)
        ss_kk = psacc(CT, T)
        for wsel, raw, ss, boff in ((wq3, qr, ss_q, 0), (wk3, kr, ss_kk, 8)):
            for mp in range(4):
                ps = psG(512)
                for mh in range(2):
                    m = 2 * mp + mh
                    half = ps[:, mh * CT:(mh + 1) * CT]
                    for k in range(8):
                        MM(half, wsel[:, k, m * 128:(m + 1) * 128],
                           h3[:, k, cols], start=(k == 0), stop=(k == 7))
                    nc.scalar.activation(raw[:, m], half, AF.Identity,
                                         bias=cst["bqkv"][:, boff + m:boff + m + 1])
            for m in range(8):
                sq = smallp.tile([128, CT], BF, tag="sq3", bufs=1)
                nc.scalar.activation(sq[:], raw[:, m], AF.Square)
                MM(ss, cst["sel16"][:, m], sq[:],
                   start=(m == 0), stop=(m == 7))

        # V rows (row-major) with bias via ones outer-product
        v_t = str1p.tile([128, 2, HD], BF, tag="v_t")
        for rc in range(2):
            for nn in range(2):
                psv = psG(512)
                MM(psv, cst["onesr"][:], cst["bqv"][:, nn * 512:(nn + 1) * 512],
                   start=True, stop=False)
                for k in range(8):
                    MM(psv,
                       h3[:, k, ti * CT + rc * 128: ti * CT + (rc + 1) * 128],
                       wv3[:, k, nn * 512:(nn + 1) * 512],
                       start=False, stop=(k == 7))
                nc.scalar.copy(v_t[:, rc, nn * 512:(nn + 1) * 512], psv)
        rope_apply(qr, ss_q, lambda mp: qr[:, 2 * mp:2 * mp + 2],
                   "cosm_q", "pm_q")
        rope_apply(kr, ss_kk, lambda mp: kr[:, 2 * mp:2 * mp + 2],
                   "cosm_k", "pm_k")

        # mouse attention: 8 positions per matmul, block-diag mask.
        # Per 128-col group: scores+exp for 16 heads, then unnormalized AV
        # into per-cc-pair psums; normalize once per (pair, group).
        den = psacc(CT, T)
        rcps = []
        for g in range(GC):
            lcols = slice(g * 128, (g + 1) * 128)
            e_tiles = []
            for h in range(H):
                cc, hh = h // 2, h % 2
                pr = slice(hh * 64, (hh + 1) * 64)
                pse = ps1(128)
                MM(pse, kr[pr, cc, lcols], qr[pr, cc, lcols],
                   start=True, stop=True)
                em = smallp.tile([128, 128], BF, tag="em", bufs=1)
                nc.vector.tensor_tensor(em[:], pse, cst["maskd"][:], TT.add)
                e = ep.tile([128, 128], BF, tag="e")
                nc.scalar.activation(e[:], em[:], AF.Exp)
                e_tiles.append(e)
                MM(den[:, lcols], cst["colsel"][:, h], e[:],
                   start=(h == 0), stop=(h == 15))
            rcp = smallp.tile([T, 128], BF, tag="rcp", name=f"rcp_{ti}_{g}")
            with nc.allow_low_precision("softmax denominators tolerate bf16"):
                nc.vector.reciprocal(rcp[:], den[:, lcols])
            for cp in range(4):       # cc pairs share a [128, 256] psum
                pst = ps1(256)
                rep = ps1(256)
                for cl in range(2):
                    cc = 2 * cp + cl
                    for hh in range(2):
                        h = 2 * cc + hh
                        MM(pst[hh * 64:(hh + 1) * 64, cl * 128:(cl + 1) * 128],
                           v_t[:, g, h * 64:(h + 1) * 64],
                           e_tiles[h][:], start=True, stop=True)
                    MM(rep[:, cl * 128:(cl + 1) * 128],
                       cst["repsel"][:, cc * 128:(cc + 1) * 128], rcp[:],
                       start=True, stop=True)
                rep_sb = smallp.tile([128, 256], BF, tag="rep_sb", bufs=1)
                nc.scalar.copy(rep_sb[:], rep)
                nc.vector.tensor_tensor(
                    attnT[:, 2 * cp:2 * cp + 2,
                          ti * CT + g * 128: ti * CT + (g + 1) * 128],
                    pst.rearrange("p (a b) -> p a b", b=128),
                    rep_sb[:].rearrange("p (a b) -> p a b", b=128), TT.mult)

    # ===== S5+S6 fused: mouse proj + residual -> partial out; kb q GEMM+rope =====
    wpm = load_w("wpm", 8, C)
    wqk = load_w("wqk", 12, HD)
    partial = dramp.tile([128, NTC, 12, CT], F32, tag="partial")
    qkT = bigp.tile([128, 8, NT], BF, tag="big")
    for ti in range(NTC):
        cols = slice(ti * CT, (ti + 1) * CT)
        xb = strp.tile([128, 12, CT], BF, tag="stream")
        dma(xb[:], dram["xtb"].ap()[:, ti])
        hsb = strp.tile([128, 12, CT], BF, tag="hsb", bufs=1)
        for mp in range(6):
            ps = psG(512)
            for mh in range(2):
                m = 2 * mp + mh
                half = ps[:, mh * CT:(mh + 1) * CT]
                for k in range(8):
                    MM(half, wpm[:, k, m * 128:(m + 1) * 128],
                       attnT[:, k, cols], start=(k == 0), stop=(k == 7))
            sl = slice(2 * mp, 2 * mp + 2)
            psv = ps.rearrange("p (a b) -> p a b", b=CT)
            nc.vector.tensor_tensor(hsb[:, sl], psv, xb[:, sl], TT.add)
            xf = smallp.tile([128, 2, CT], F32, tag="istage")
            dma(xf[:], dram["xtf"].ap()[:, ti, sl])
            of = smallp.tile([128, 2, CT], F32, tag="ostage", bufs=1)
            nc.vector.tensor_tensor(of[:], psv, xf[:], TT.add)
            dma(partial[:, ti, sl], of[:])
        qr = str1p.tile([128, 8, CT], BF, tag="qraw", bufs=2)
        ss_q = psacc(CT, T)
        for mp in range(4):
            ps = psG(512)
            for mh in range(2):
                m = 2 * mp + mh
                half = ps[:, mh * CT:(mh + 1) * CT]
                for k in range(12):
                    MM(half, wqk[:, k, m * 128:(m + 1) * 128], hsb[:, k],
                       start=(k == 0), stop=(k == 11))
                nc.scalar.copy(qr[:, m], half)
        for m in range(8):
            sq = smallp.tile([128, CT], BF, tag="sq3", bufs=1)
            nc.scalar.activation(sq[:], qr[:, m], AF.Square)
            MM(ss_q, cst["sel16"][:, m], sq[:], start=(m == 0), stop=(m == 7))
        rope_apply(qr, ss_q, lambda mp: qkT[:, 2 * mp:2 * mp + 2, cols],
                   "cosk_q", "pk_q")

    # ================= S7: keyboard attention =================
    akT = bigp.tile([128, 8, NT], BF, tag="big")
    for ti in range(NTC):
        cols = slice(ti * CT, (ti + 1) * CT)
        for cp in range(4):
            pst = ps1(512)
            den = psacc(CT, 4)
            for cl in range(2):
                cc = 2 * cp + cl
                for hh in range(2):
                    h = 2 * cc + hh
                    pr = slice(hh * 64, (hh + 1) * 64)
                    pse = ps1(CT, T)
                    MM(pse, kkT[pr, cc, :], qkT[pr, cc, cols],
                       start=True, stop=True)
                    em = smallp.tile([T, CT], BF, tag="em_kb", bufs=1)
                    nc.vector.tensor_tensor(
                        em[:].rearrange("p (s t) -> p s t", t=T),
                        pse.rearrange("p (s t) -> p s t", t=T),
                        cst["maskk"][:16, None, :].to_broadcast([T, PC, T]),
                        TT.add)
                    e = ekbp.tile([T, CT], BF, tag="e_kb")
                    nc.scalar.activation(e[:], em[:], AF.Exp)
                    MM(den, cst["colsel16"][:, h, 4 * cp:4 * cp + 4], e[:],
                       start=(h == 4 * cp), stop=(h == 4 * cp + 3))
                    MM(pst[pr, cl * CT:(cl + 1) * CT],
                       vk[:, h * 64:(h + 1) * 64], e[:],
                       start=True, stop=True)
            rcp = smallp.tile([4, CT], BF, tag="rcp4", name=f"rcp4_{ti}_{cp}")
            with nc.allow_low_precision("softmax denominators tolerate bf16"):
                nc.vector.reciprocal(rcp[:], den)
            rep = ps1(512)
            for cl in range(2):
                MM(rep[:, cl * CT:(cl + 1) * CT],
                   cst["repsel4"][:, cl * 128:(cl + 1) * 128], rcp[:],
                   start=True, stop=True)
            rep_sb = smallp.tile([128, 512], BF, tag="rep_sb", bufs=1)
            nc.scalar.copy(rep_sb[:], rep)
            nc.vector.tensor_tensor(
                akT[:, 2 * cp:2 * cp + 2, cols],
                pst.rearrange("p (a b) -> p a b", b=CT),
                rep_sb[:].rearrange("p (a b) -> p a b", b=CT), TT.mult)

    # ================= S8: kb proj + final accumulate -> out =================
    wpk = load_w("wpk", 8, C)
    for ti in range(NTC):
        cols = slice(ti * CT, (ti + 1) * CT)
        for mp in range(6):
            ps = psG(512)
            for mh in range(2):
                m = 2 * mp + mh
                half = ps[:, mh * CT:(mh + 1) * CT]
                for k in range(8):
                    MM(half, wpk[:, k, m * 128:(m + 1) * 128], akT[:, k, cols],
                       start=(k == 0), stop=(k == 7))
            sl = slice(2 * mp, 2 * mp + 2)
            pf = smallp.tile([128, 2, CT], F32, tag="istage")
            dma(pf[:], partial[:, ti, sl])
            ob = smallp.tile([128, 2, CT], F32, tag="ostage", bufs=1)
            nc.vector.tensor_tensor(ob[:], ps.rearrange("p (a b) -> p a b", b=CT),
                                    pf[:], TT.add)
            dma(out_ext.ap()[:, ti, sl], ob[:])

    for p in reversed(ctxs):
        p.__exit__(None, None, None)


# ------------------------------------------------------------------ entry
def kernel(**inputs):
    per_core = _prep_host(inputs)
    if "graph" not in _CACHE:
        _CACHE["graph"] = build_graph(per_core[0])
    nc = _CACHE["graph"]
    res = run_bass_kernel_spmd(nc, per_core, core_ids=list(range(NCORES)))
    outs = []
    for i in range(NCORES):
        o = np.asarray(res.results[i]["out"])          # [128, NTC, 12, CT]
        o = o.transpose(2, 0, 1, 3).reshape(C, NSC, T)  # [C, s_local, t]
        outs.append(o)
    full = np.concatenate(outs, axis=1)[:, :S, :]       # [C, 880, 16]
    out = np.ascontiguousarray(np.transpose(full, (2, 1, 0)))
    return out.reshape(1, T * S, C).astype(np.float32)


# revision 29
# speedup vs baseline: 1.2053x; 1.2053x over previous
"""Trainium2 Bass kernel for nn_ActionModuleTorch (sparse action attention).

Sharding: data-parallel over spatial positions S=880 (padded to 896 -> 112
positions per core on 8 cores), no collectives. Weights replicated, host-cast
to bf16. Activations are feature-major ([C on partitions, (s,t) tokens on the
free dim, frame t minor]) so all six big GEMMs chain with natural-layout bf16
weights as the stationary matmul operand.

Cross-partition reductions (layernorm / rmsnorm / softmax denominators) use
ones/selector matmuls on the PE; per-token statistics are broadcast back
across partitions with outer-product matmuls. Mouse attention batches 8
positions per matmul with a block-diagonal mask (off-diagonal cross-position
scores are killed by exp(-30000) -> 0, which also makes the softmax
denominator a single ones-reduction). V is produced row-major by swapping the
matmul operand roles so attention output lands feature-major for the
projection GEMMs.
"""

import sys

import numpy as np

sys.path.insert(0, "/opt/trn_rl_repo")

import ml_dtypes  # noqa: E402

import concourse.bass as bass  # noqa: E402
import concourse.mybir as mybir  # noqa: E402
from concourse import bacc  # noqa: E402
from concourse import tile_utils  # noqa: E402
from concourse.bass_utils import run_bass_kernel_spmd  # noqa: E402
from concourse.tile import TileContext  # noqa: E402

tile_utils.max_sbuf_usage = 212 * 1024  # stale 192KiB default; 224KiB phys

BF = mybir.dt.bfloat16
F32 = mybir.dt.float32
bf16 = ml_dtypes.bfloat16

T = 16
S = 880
SP = 896
NCORES = 8
NSC = SP // NCORES        # 112 positions/core
NT = NSC * T              # 1792 tokens/core
C = 1536
H, D, HD, MHD = 16, 64, 1024, 1024
THETA, T_DIM = 256.0, 8
RATIO, WIN, LOCAL = 4, 3, 6
FW = RATIO * WIN          # 12

CT = 256                  # tokens per chunk (16 positions)
PC = CT // T              # 16 positions per chunk
NTC = NT // CT            # 7 chunks per core
GC = 2                    # 128-col attention groups per chunk

_CACHE = {}


# ---------------------------------------------------------------- host tables
def _rope_tables():
    freqs = 1.0 / (THETA ** (np.arange(0, T_DIM, 2, dtype=np.float64) / T_DIM))
    ang = np.arange(T, dtype=np.float64)[:, None] * freqs[None, :]
    cos = np.concatenate([np.repeat(np.cos(ang), 2, 1), np.ones((T, D - T_DIM))], 1)
    sin = np.concatenate([np.repeat(np.sin(ang), 2, 1), np.zeros((T, D - T_DIM))], 1)
    return cos.astype(np.float32), sin.astype(np.float32)  # [t, d]


def _perm_rot(w, scale):
    """lhsT [128,128] for rot = P @ x per 64-head-block, weight+scale folded."""
    P = np.zeros((D, D), np.float32)
    for i in range(T_DIM // 2):
        P[2 * i, 2 * i + 1] = -1.0
        P[2 * i + 1, 2 * i] = 1.0
    PwT = (P * (w[None, :] * scale)).T  # [d, j]
    out = np.zeros((128, 128), np.float32)
    out[:D, :D] = PwT
    out[D:, D:] = PwT
    return out


def _cos_table(cos, w, scale):
    m = (cos * (w[None, :] * scale)).T  # [d, t]
    return np.concatenate([m, m], axis=0)  # [128, 16]


def _consts(inp):
    cos, sin = _rope_tables()
    a = {}
    a["sint"] = np.concatenate([sin.T, sin.T], axis=0)
    a["cosm_q"] = _cos_table(cos, np.asarray(inp["qn_m"], np.float32), 0.125)
    a["cosm_k"] = _cos_table(cos, np.asarray(inp["kn_m"], np.float32), 1.0)
    a["cosk_q"] = _cos_table(cos, np.asarray(inp["qn_k"], np.float32), 0.125)
    a["cosk_k"] = _cos_table(cos, np.asarray(inp["kn_k"], np.float32), 1.0)
    a["pm_q"] = _perm_rot(np.asarray(inp["qn_m"], np.float32), 0.125)
    a["pm_k"] = _perm_rot(np.asarray(inp["kn_m"], np.float32), 1.0)
    a["pk_q"] = _perm_rot(np.asarray(inp["qn_k"], np.float32), 0.125)
    a["pk_k"] = _perm_rot(np.asarray(inp["kn_k"], np.float32), 1.0)

    i = np.arange(T)
    mask = (i[None, :] <= i[:, None]) & (i[:, None] - i[None, :] < LOCAL)
    mbT = np.where(mask.T, 0.0, -30000.0).astype(np.float32)  # [t', t]
    md = np.full((128, 128), -30000.0, np.float32)
    for g in range(8):
        md[g * T:(g + 1) * T, g * T:(g + 1) * T] = mbT
    a["maskd"] = md                       # mouse: block-diag over 8 positions
    a["maskk"] = np.tile(mbT, (8, 1))     # kb: 8 head-blocks stacked

    # sel16[p, m, c] = 1 iff c == 2m + p//64 : per-chunk head sumsq selector
    sel16 = np.zeros((128, 8, 16), np.float32)
    for m in range(8):
        sel16[:64, m, 2 * m] = 1.0
        sel16[64:, m, 2 * m + 1] = 1.0
    a["sel16"] = sel16
    # colsel[p, h, c] = 1 iff c == h : full-column-sum into row h
    colsel = np.zeros((128, 16, 16), np.float32)
    for h in range(16):
        colsel[:, h, h] = 1.0
    a["colsel"] = colsel
    a["colsel16"] = colsel[:16].copy()  # [16, 16, 16] for kb denominators
    a["ones128"] = np.ones((128, 1), np.float32)
    a["onesr"] = np.ones((1, 128), np.float32)
    repsel = np.zeros((16, 1024), np.float32)
    for cc in range(8):
        for hh in range(2):
            repsel[2 * cc + hh, cc * 128 + hh * 64: cc * 128 + (hh + 1) * 64] = 1.0
    a["repsel"] = repsel
    repsel4 = np.zeros((4, 256), np.float32)
    for cl in range(2):
        for hh in range(2):
            repsel4[2 * cl + hh, cl * 128 + hh * 64: cl * 128 + (hh + 1) * 64] = 1.0
    a["repsel4"] = repsel4
    return a


def _pack_w(w):
    """[K, M] -> [128, K//128, M] (partition-major for direct DMA)."""
    K, M = w.shape
    return np.ascontiguousarray(w.reshape(K // 128, 128, M).transpose(1, 0, 2))


def _pack_b(v):
    """[C] -> [128, C//128]."""
    return np.ascontiguousarray(v.reshape(-1, 128).T)


def _prep_host(inp):
    f = {k: np.asarray(v, np.float32) for k, v in inp.items()
         if k not in ("tt", "th", "tw")}
    sh = {}
    mm_w1 = f["mm_w1"]
    sh["w1"] = _pack_w(mm_w1[:C]).astype(bf16)
    wg = np.zeros((128, MHD), np.float32)
    wg[:FW * 2] = mm_w1[C:]
    sh["wg"] = wg.astype(bf16)
    sh["b1"] = _pack_b(f["mm_b1"])
    sh["w2"] = _pack_w(f["mm_w2"]).astype(bf16)
    sh["b2"] = _pack_b(f["mm_b2"])

    wqkv = f["mm_ln_g"][:, None] * f["t_qkv_w"]
    bqkv = f["mm_ln_b"] @ f["t_qkv_w"]
    sh["wq3"] = _pack_w(wqkv[:, :HD]).astype(bf16)
    sh["wk3"] = _pack_w(wqkv[:, HD:2 * HD]).astype(bf16)
    sh["wv3"] = _pack_w(wqkv[:, 2 * HD:]).astype(bf16)
    sh["bqkv"] = _pack_b(bqkv[:2 * HD])          # [128, 16] q|k biases
    sh["bqv"] = bqkv[2 * HD:].astype(bf16)[None, :]  # [1, 1024]
    sh["wpm"] = _pack_w(f["proj_mouse_w"]).astype(bf16)
    sh["wqk"] = _pack_w(f["mouse_q_w"]).astype(bf16)
    sh["wkv"] = _pack_w(f["kb_kv_w"]).astype(bf16)
    sh["wpk"] = _pack_w(f["proj_kb_w"]).astype(bf16)

    kbin = np.zeros((128, 72), np.float32)
    kbin[:6] = f["keyboard_condition"][0].T
    sh["kbin"] = kbin.astype(bf16)
    kw1 = np.zeros((128, 128), np.float32)
    kw1[:6] = f["kb_w1"]
    sh["kw1"] = kw1.astype(bf16)
    sh["kb1"] = f["kb_b1"].reshape(128, 1).copy()
    sh["kw2"] = f["kb_w2"].astype(bf16)
    sh["kb2"] = f["kb_b2"].reshape(128, 1).copy()

    mc = f["mouse_condition"][0]
    gm24 = np.stack([mc[4 * t:4 * t + FW].reshape(-1) for t in range(T)])
    gm24T = np.zeros((128, T), np.float32)
    gm24T[:FW * 2] = gm24.T
    sh["gm24"] = gm24T.astype(bf16)

    for k, v in _consts(inp).items():
        sh[k] = v.astype(np.float32) if k in ("maskd", "maskk") else v.astype(bf16)

    # x shard, feature-major, chunk-major DMA layout [128, 14, 12, 128]
    x = f["x"][0].reshape(T, S, C)
    xt = np.transpose(x, (2, 1, 0))                      # [C, s, t]
    xt_pad = np.zeros((C, SP, T), np.float32)
    xt_pad[:, :S] = xt
    per_core = []
    for c in range(NCORES):
        sl = xt_pad[:, c * NSC:(c + 1) * NSC, :].reshape(C, NT)
        # [C, NT] -> [128, NTC, 12, CT]
        arr = sl.reshape(12, 128, NTC, CT).transpose(1, 2, 0, 3)
        m = dict(sh)
        m["xtf"] = np.ascontiguousarray(arr)
        m["xtb"] = m["xtf"].astype(bf16)
        per_core.append(m)
    return per_core


# ------------------------------------------------------------------ builder
def _declare(nc, name, arr):
    dt = BF if arr.dtype == bf16 else F32
    return nc.declare_dram_parameter(name, list(arr.shape), dt, isOutput=False)


def build_graph(example_map):
    nc = bacc.Bacc()
    dram = {k: _declare(nc, k, v) for k, v in example_map.items()}
    out_ext = nc.declare_dram_parameter("out", [128, NTC, 12, CT], F32,
                                        isOutput=True)
    with TileContext(nc) as tc:
        _body(nc, tc, dram, out_ext)
    nc.finalize()
    return nc


def _body(nc, tc, dram, out_ext):
    TT = mybir.AluOpType
    AF = mybir.ActivationFunctionType
    MM = nc.tensor.matmul

    ctxs = []

    def pool(name, bufs, space="SBUF"):
        p = tc.tile_pool(name=name, bufs=bufs, space=space)
        ctxs.append(p)
        return p.__enter__()

    constp = pool("const", 1)
    bigp = pool("big", 2)       # whole-core activations, rotating slots
    wtsp = pool("wts", 2)       # streamed weights (24KB class)
    wts3p = pool("wts3", 1)     # v-weights slot during fused S3/S4 (16KB)
    strp = pool("stream", 2)    # xtb / hsb staging
    str1p = pool("stream1", 1)  # per-chunk single-buffer staging
    smallp = pool("small", 2)   # small per-(pair,chunk) temporaries
    ep = pool("eprobs", 17)     # mouse attention prob tiles
    ekbp = pool("ekb", 4)      # kb attention prob tiles
    ppG = pool("psG", 3, space="PSUM")
    pp1 = pool("ps1", 3, space="PSUM")
    ppA = pool("psacc", 2, space="PSUM")
    dramp = pool("drambounce", 1, space="DRAM")

    cnt = [0]

    def ps1(pcols=512, prow=128):
        cnt[0] += 1
        t = pp1.tile([128, 512], F32, tag="ps1", name=f"ps1_{cnt[0]}")
        return t[:prow, :pcols]

    def psG(pcols=512, prow=128):
        cnt[0] += 1
        t = ppG.tile([128, 512], F32, tag="psG", name=f"psG_{cnt[0]}")
        return t[:prow, :pcols]

    def psacc(pcols=512, prow=128):
        cnt[0] += 1
        t = ppA.tile([128, 512], F32, tag="psacc", name=f"psacc_{cnt[0]}")
        return t[:prow, :pcols]

    def dma(dst, src):
        nc.sync.dma_start(out=dst, in_=src)

    cst = {}
    for nm in ["cosm_q", "cosm_k", "cosk_q", "cosk_k", "sint", "pm_q", "pm_k",
               "pk_q", "pk_k", "maskd", "maskk", "sel16", "colsel", "colsel16",
               "ones128", "onesr", "repsel", "gm24", "wg", "kbin", "kw1", "kw2",
               "bqv", "b1", "b2", "bqkv", "kb1", "kb2", "repsel4"]:
        h = dram[nm]
        t = constp.tile(list(h.shape), h.dtype, tag=f"c_{nm}")
        dma(t[:], h.ap())
        cst[nm] = t

    def load_w(name, kchunks, mcols, pl=wtsp, tag="wts"):
        t = pl.tile([128, kchunks, mcols], BF, tag=tag)
        dma(t[:], dram[name].ap())
        return t

    # ---- G = wg^T @ gm24 + b1 : per-frame GEMM1 bias [1024, 16] ----
    g_sb = constp.tile([128, 8, T], F32, tag="g_sb")
    for m in range(8):
        ps = ps1(T)
        MM(ps, cst["wg"][:, m * 128:(m + 1) * 128], cst["gm24"][:],
           start=True, stop=True)
        nc.vector.tensor_tensor(g_sb[:, m], ps,
                                cst["b1"][:, m:m + 1].to_broadcast([128, T]),
                                TT.add)

    # ---- keyboard K/V (tiny, once per core) ----
    ps = ps1(72)
    MM(ps, cst["kw1"][:], cst["kbin"][:], start=True, stop=True)
    kc1 = smallp.tile([128, 72], F32, tag="kc1")
    nc.vector.tensor_tensor(kc1[:], ps, cst["kb1"][:].to_broadcast([128, 72]),
                            TT.add)
    kc1b = smallp.tile([128, 72], BF, tag="kc1b")
    nc.scalar.activation(kc1b[:], kc1[:], AF.Silu)
    ps = ps1(72)
    MM(ps, cst["kw2"][:], kc1b[:], start=True, stop=True)
    kc2 = constp.tile([128, 72], BF, tag="kc2")
    nc.vector.tensor_tensor(kc2[:], ps, cst["kb2"][:].to_broadcast([128, 72]),
                            TT.add)
    kc2v = kc2[:].rearrange("p (t r) -> p r t", r=4)  # stride-4 window view

    def gk_rhs(j):
        return kc2v[:, j % 4, j // 4: j // 4 + T]  # [128, 16] = gk^T chunk j

    wkv_k = wtsp.tile([128, 12, HD], BF, tag="wts", name="wkv_k")
    dma(wkv_k[:], dram["wkv"].ap()[:, :, :HD])
    kkT = constp.tile([128, 8, T], BF, tag="kkT")     # roped keyboard K^T
    kk_raw = constp.tile([128, 8, T], BF, tag="kk_raw")
    ss_k = psacc(T, T)
    for m in range(8):
        ps = ps1(T)
        for j in range(FW):
            MM(ps, wkv_k[:, j, m * 128:(m + 1) * 128], gk_rhs(j),
               start=(j == 0), stop=(j == FW - 1))
        nc.scalar.copy(kk_raw[:, m], ps)
        sq = smallp.tile([128, T], BF, tag="sq_kk")
        nc.scalar.activation(sq[:], kk_raw[:, m], AF.Square)
        MM(ss_k, cst["sel16"][:, m], sq[:], start=(m == 0), stop=(m == 7))
    rstd_k = smallp.tile([T, T], BF, tag="rstd_k")
    tmp = smallp.tile([T, T], F32, tag="t_rstdk")
    nc.vector.tensor_scalar(tmp[:], ss_k, 1.0 / 64, 1e-6, TT.mult, TT.add)
    nc.vector.reciprocal(tmp[:], tmp[:])
    nc.scalar.activation(rstd_k[:], tmp[:], AF.Sqrt)
    for m in range(8):
        rep = ps1(T)
        MM(rep, cst["repsel"][:, m * 128:(m + 1) * 128], rstd_k[:],
           start=True, stop=True)
        rot = ps1(T)
        MM(rot, cst["pk_k"][:], kk_raw[:, m], start=True, stop=True)
        t1 = smallp.tile([128, T], F32, tag="t1_kk")
        nc.vector.tensor_tensor(t1[:], kk_raw[:, m], cst["cosk_k"][:], TT.mult)
        t2 = smallp.tile([128, T], F32, tag="t2_kk")
        nc.vector.tensor_tensor(t2[:], rot, cst["sint"][:], TT.mult)
        nc.vector.tensor_tensor(t1[:], t1[:], t2[:], TT.add)
        nc.vector.tensor_tensor(kkT[:, m], t1[:], rep, TT.mult)
    wkv_v = wtsp.tile([128, 12, HD], BF, tag="wts", name="wkv_v")
    dma(wkv_v[:], dram["wkv"].ap()[:, :, HD:])
    vk = constp.tile([T, HD], BF, tag="vk")  # keyboard V rows [16, 1024]
    for nn in range(2):
        psv = psacc(512, T)
        for j in range(FW):
            MM(psv, gk_rhs(j), wkv_v[:, j, nn * 512:(nn + 1) * 512],
               start=(j == 0), stop=(j == FW - 1))
        nc.scalar.copy(vk[:, nn * 512:(nn + 1) * 512], psv)

    # ================= S1: GEMM1 x(+windowed mouse) -> h1 =================
    w1 = load_w("w1", 12, MHD)
    h1 = bigp.tile([128, 8, NT], BF, tag="big")
    for ti in range(NTC):
        xb = strp.tile([128, 12, CT], BF, tag="stream")
        dma(xb[:], dram["xtb"].ap()[:, ti])
        for mp in range(4):           # pairs of m-chunks share one psum bank
            ps = psG(512)
            for mh in range(2):
                m = 2 * mp + mh
                half = ps[:, mh * CT:(mh + 1) * CT]
                for k in range(12):
                    MM(half, w1[:, k, m * 128:(m + 1) * 128], xb[:, k],
                       start=(k == 0), stop=(k == 11))
                h3v = half.rearrange("p (s t) -> p s t", t=T)
                nc.vector.tensor_tensor(
                    h3v, h3v, g_sb[:, m, None, :].to_broadcast([128, PC, T]),
                    TT.add)
            nc.scalar.activation(
                h1[:, 2 * mp:2 * mp + 2, ti * CT:(ti + 1) * CT], ps,
                AF.Gelu_apprx_tanh)

    # ================= S2: GEMM2 + LayerNorm -> h3 =================
    w2 = load_w("w2", 8, MHD)
    h3 = bigp.tile([128, 8, NT], BF, tag="big")
    for ti in range(NTC):
        h2b = str1p.tile([128, 8, CT], BF, tag="h2b")
        st_mu = psacc(CT, 1)
        st_sq = psacc(CT, 1)
        for mp in range(4):
            ps = psG(512)
            for mh in range(2):
                m = 2 * mp + mh
                half = ps[:, mh * CT:(mh + 1) * CT]
                for k in range(8):
                    MM(half, w2[:, k, m * 128:(m + 1) * 128],
                       h1[:, k, ti * CT:(ti + 1) * CT],
                       start=(k == 0), stop=(k == 7))
                nc.scalar.activation(h2b[:, m], half, AF.Identity,
                                     bias=cst["b2"][:, m:m + 1])
        for m in range(8):
            sq = smallp.tile([128, CT], BF, tag="sq2")
            nc.scalar.activation(sq[:], h2b[:, m], AF.Square)
            MM(st_mu, cst["ones128"][:], h2b[:, m],
               start=(m == 0), stop=(m == 7))
            MM(st_sq, cst["ones128"][:], sq[:],
               start=(m == 0), stop=(m == 7))
        mu = smallp.tile([1, CT], F32, tag="mu")
        nc.vector.tensor_scalar(mu[:], st_mu, 1.0 / MHD, 0.0, TT.mult, TT.add)
        mu2 = smallp.tile([1, CT], F32, tag="mu2")
        nc.vector.tensor_tensor(mu2[:], mu[:], mu[:], TT.mult)
        var = smallp.tile([1, CT], F32, tag="ln_var")
        nc.vector.tensor_scalar(var[:], st_sq, 1.0 / MHD, 1e-5,
                                TT.mult, TT.add)
        nc.vector.tensor_tensor(var[:], var[:], mu2[:], TT.subtract)
        nc.vector.reciprocal(var[:], var[:])
        rstd = smallp.tile([1, CT], BF, tag="rstd2")
        nc.scalar.activation(rstd[:], var[:], AF.Sqrt)
        mub = smallp.tile([1, CT], BF, tag="mub")
        nc.vector.tensor_copy(mub[:], mu[:])
        rep_mu = ps1(CT)
        MM(rep_mu, cst["onesr"][:], mub[:], start=True, stop=True)
        rep_rs = ps1(CT)
        MM(rep_rs, cst["onesr"][:], rstd[:], start=True, stop=True)
        rs_sb = smallp.tile([128, CT], BF, tag="rs_sb")
        nc.scalar.copy(rs_sb[:], rep_rs)
        for mp in range(4):
            sl = slice(2 * mp, 2 * mp + 2)
            tmp = smallp.tile([128, 2, CT], BF, tag="ln_t", bufs=1)
            nc.vector.tensor_tensor(
                tmp[:], h2b[:, sl],
                rep_mu[:, None, :].to_broadcast([128, 2, CT]), TT.subtract)
            nc.vector.tensor_tensor(
                h3[:, sl, ti * CT:(ti + 1) * CT], tmp[:],
                rs_sb[:, None, :].to_broadcast([128, 2, CT]), TT.mult)

    # ---- shared rms+rope epilogue, pair-packed ----
    def rope_apply(raw_sb, ss, out_ap, cosname, pname):
        rstd = smallp.tile([T, CT], BF, tag="rstd3")
        tmp = smallp.tile([T, CT], F32, tag="t_rstd3")
        nc.vector.tensor_scalar(tmp[:], ss, 1.0 / 64, 1e-6, TT.mult, TT.add)
        nc.vector.reciprocal(tmp[:], tmp[:])
        nc.scalar.activation(rstd[:], tmp[:], AF.Sqrt)
        for mp in range(4):
            rep = ps1(512)
            rot = ps1(512)
            for mh in range(2):
                m = 2 * mp + mh
                MM(rep[:, mh * CT:(mh + 1) * CT],
                   cst["repsel"][:, m * 128:(m + 1) * 128], rstd[:],
                   start=True, stop=True)
                MM(rot[:, mh * CT:(mh + 1) * CT], cst[pname][:], raw_sb[:, m],
                   start=True, stop=True)
            t1 = smallp.tile([128, 512], BF, tag="t1_r", bufs=1)
            nc.vector.tensor_tensor(
                t1[:].rearrange("p (s t) -> p s t", t=T),
                raw_sb[:, 2 * mp:2 * mp + 2].rearrange("p a b -> p (a b)")
                .rearrange("p (s t) -> p s t", t=T),
                cst[cosname][:, None, :].to_broadcast([128, 2 * PC, T]),
                TT.mult)
            t2 = smallp.tile([128, 512], BF, tag="t2_r", bufs=1)
            nc.vector.tensor_tensor(
                t2[:].rearrange("p (s t) -> p s t", t=T),
                rot.rearrange("p (s t) -> p s t", t=T),
                cst["sint"][:, None, :].to_broadcast([128, 2 * PC, T]),
                TT.mult)
            nc.vector.tensor_tensor(t1[:], t1[:], t2[:], TT.add)
            nc.vector.tensor_tensor(out_ap(mp),
                                    t1[:].rearrange("p (a b) -> p a b", b=CT),
                                    rep.rearrange("p (a b) -> p a b", b=CT),
                                    TT.mult)

    # ============ S3+S4 fused: qkv GEMM + rms/rope + mouse attention ============
    wq3 = load_w("wq3", 8, HD)
    wk3 = load_w("wk3", 8, HD)
    wv3 = load_w("wv3", 8, HD, pl=wts3p, tag="wts3")
    attnT = bigp.tile([128, 8, NT], BF, tag="big")
    for ti in range(NTC):
        cols = slice(ti * CT, (ti + 1) * CT)
        qr = str1p.tile([128, 8, CT], BF, tag="qraw")
        kr = str1p.tile([128, 8, CT], BF, tag="kraw")
        ss_q = psacc(CT, T)
        ss_kk = psacc(CT, T)
        for wsel, raw, ss, boff in ((wq3, qr, ss_q, 0), (wk3, kr, ss_kk, 8)):
            for mp in range(4):
                ps = psG(512)
                for mh in range(2):
                    m = 2 * mp + mh
                    half = ps[:, mh * CT:(mh + 1) * CT]
                    for k in range(8):
                        MM(half, wsel[:, k, m * 128:(m + 1) * 128],
                           h3[:, k, cols], start=(k == 0), stop=(k == 7))
                    nc.scalar.activation(raw[:, m], half, AF.Identity,
                                         bias=cst["bqkv"][:, boff + m:boff + m + 1])
            for m in range(8):
                sq = smallp.tile([128, CT], BF, tag="sq3")
                nc.scalar.activation(sq[:], raw[:, m], AF.Square)
                MM(ss, cst["sel16"][:, m], sq[:],
                   start=(m == 0), stop=(m == 7))

        # V rows (row-major) with bias via ones outer-product
        v_t = str1p.tile([128, 2, HD], BF, tag="v_t")
        for rc in range(2):
            for nn in range(2):
                psv = psG(512)
                MM(psv, cst["onesr"][:], cst["bqv"][:, nn * 512:(nn + 1) * 512],
                   start=True, stop=False)
                for k in range(8):
                    MM(psv,
                       h3[:, k, ti * CT + rc * 128: ti * CT + (rc + 1) * 128],
                       wv3[:, k, nn * 512:(nn + 1) * 512],
                       start=False, stop=(k == 7))
                nc.scalar.copy(v_t[:, rc, nn * 512:(nn + 1) * 512], psv)
        rope_apply(qr, ss_q, lambda mp: qr[:, 2 * mp:2 * mp + 2],
                   "cosm_q", "pm_q")
        rope_apply(kr, ss_kk, lambda mp: kr[:, 2 * mp:2 * mp + 2],
                   "cosm_k", "pm_k")

        # mouse attention: 8 positions per matmul, block-diag mask.
        # Per 128-col group: scores+exp for 16 heads, then unnormalized AV
        # into per-cc-pair psums; normalize once per (pair, group).
        den = psacc(CT, T)
        rcps = []
        for g in range(GC):
            lcols = slice(g * 128, (g + 1) * 128)
            e_tiles = []
            for h in range(H):
                cc, hh = h // 2, h % 2
                pr = slice(hh * 64, (hh + 1) * 64)
                pse = ps1(128)
                MM(pse, kr[pr, cc, lcols], qr[pr, cc, lcols],
                   start=True, stop=True)
                em = smallp.tile([128, 128], BF, tag="em", bufs=1)
                nc.vector.tensor_tensor(em[:], pse, cst["maskd"][:], TT.add)
                e = ep.tile([128, 128], BF, tag="e")
                nc.scalar.activation(e[:], em[:], AF.Exp)
                e_tiles.append(e)
                MM(den[:, lcols], cst["colsel"][:, h], e[:],
                   start=(h == 0), stop=(h == 15))
            rcp = smallp.tile([T, 128], BF, tag="rcp", name=f"rcp_{ti}_{g}")
            with nc.allow_low_precision("softmax denominators tolerate bf16"):
                nc.vector.reciprocal(rcp[:], den[:, lcols])
            for cp in range(4):       # cc pairs share a [128, 256] psum
                pst = ps1(256)
                rep = ps1(256)
                for cl in range(2):
                    cc = 2 * cp + cl
                    for hh in range(2):
                        h = 2 * cc + hh
                        MM(pst[hh * 64:(hh + 1) * 64, cl * 128:(cl + 1) * 128],
                           v_t[:, g, h * 64:(h + 1) * 64],
                           e_tiles[h][:], start=True, stop=True)
                    MM(rep[:, cl * 128:(cl + 1) * 128],
                       cst["repsel"][:, cc * 128:(cc + 1) * 128], rcp[:],
                       start=True, stop=True)
                rep_sb = smallp.tile([128, 256], BF, tag="rep_sb")
                nc.scalar.copy(rep_sb[:], rep)
                nc.vector.tensor_tensor(
                    attnT[:, 2 * cp:2 * cp + 2,
                          ti * CT + g * 128: ti * CT + (g + 1) * 128],
                    pst.rearrange("p (a b) -> p a b", b=128),
                    rep_sb[:].rearrange("p (a b) -> p a b", b=128), TT.mult)

    # ===== S5+S6 fused: mouse proj + residual -> partial out; kb q GEMM+rope =====
    wpm = load_w("wpm", 8, C)
    wqk = load_w("wqk", 12, HD)
    partial = dramp.tile([128, NTC, 12, CT], F32, tag="partial")
    qkT = bigp.tile([128, 8, NT], BF, tag="big")
    for ti in range(NTC):
        cols = slice(ti * CT, (ti + 1) * CT)
        xb = strp.tile([128, 12, CT], BF, tag="stream")
        dma(xb[:], dram["xtb"].ap()[:, ti])
        hsb = strp.tile([128, 12, CT], BF, tag="hsb", bufs=1)
        for mp in range(6):
            ps = psG(512)
            for mh in range(2):
                m = 2 * mp + mh
                half = ps[:, mh * CT:(mh + 1) * CT]
                for k in range(8):
                    MM(half, wpm[:, k, m * 128:(m + 1) * 128],
                       attnT[:, k, cols], start=(k == 0), stop=(k == 7))
            sl = slice(2 * mp, 2 * mp + 2)
            psv = ps.rearrange("p (a b) -> p a b", b=CT)
            nc.vector.tensor_tensor(hsb[:, sl], psv, xb[:, sl], TT.add)
            xf = smallp.tile([128, 2, CT], F32, tag="istage")
            dma(xf[:], dram["xtf"].ap()[:, ti, sl])
            of = smallp.tile([128, 2, CT], F32, tag="ostage")
            nc.vector.tensor_tensor(of[:], psv, xf[:], TT.add)
            dma(partial[:, ti, sl], of[:])
        qr = str1p.tile([128, 8, CT], BF, tag="qraw")
        ss_q = psacc(CT, T)
        for mp in range(4):
            ps = psG(512)
            for mh in range(2):
                m = 2 * mp + mh
                half = ps[:, mh * CT:(mh + 1) * CT]
                for k in range(12):
                    MM(half, wqk[:, k, m * 128:(m + 1) * 128], hsb[:, k],
                       start=(k == 0), stop=(k == 11))
                nc.scalar.copy(qr[:, m], half)
        for m in range(8):
            sq = smallp.tile([128, CT], BF, tag="sq3")
            nc.scalar.activation(sq[:], qr[:, m], AF.Square)
            MM(ss_q, cst["sel16"][:, m], sq[:], start=(m == 0), stop=(m == 7))
        rope_apply(qr, ss_q, lambda mp: qkT[:, 2 * mp:2 * mp + 2, cols],
                   "cosk_q", "pk_q")

    # ================= S7: keyboard attention =================
    akT = bigp.tile([128, 8, NT], BF, tag="big")
    for ti in range(NTC):
        cols = slice(ti * CT, (ti + 1) * CT)
        for cp in range(4):
            pst = ps1(512)
            den = psacc(CT, 4)
            for cl in range(2):
                cc = 2 * cp + cl
                for hh in range(2):
                    h = 2 * cc + hh
                    pr = slice(hh * 64, (hh + 1) * 64)
                    pse = ps1(CT, T)
                    MM(pse, kkT[pr, cc, :], qkT[pr, cc, cols],
                       start=True, stop=True)
                    em = smallp.tile([T, CT], BF, tag="em_kb", bufs=1)
                    nc.vector.tensor_tensor(
                        em[:].rearrange("p (s t) -> p s t", t=T),
                        pse.rearrange("p (s t) -> p s t", t=T),
                        cst["maskk"][:16, None, :].to_broadcast([T, PC, T]),
                        TT.add)
                    e = ekbp.tile([T, CT], BF, tag="e_kb")
                    nc.scalar.activation(e[:], em[:], AF.Exp)
                    MM(den, cst["colsel16"][:, h, 4 * cp:4 * cp + 4], e[:],
                       start=(h == 4 * cp), stop=(h == 4 * cp + 3))
                    MM(pst[pr, cl * CT:(cl + 1) * CT],
                       vk[:, h * 64:(h + 1) * 64], e[:],
                       start=True, stop=True)
            rcp = smallp.tile([4, CT], BF, tag="rcp4", name=f"rcp4_{ti}_{cp}")
            with nc.allow_low_precision("softmax denominators tolerate bf16"):
                nc.vector.reciprocal(rcp[:], den)
            rep = ps1(512)
            for cl in range(2):
                MM(rep[:, cl * CT:(cl + 1) * CT],
                   cst["repsel4"][:, cl * 128:(cl + 1) * 128], rcp[:],
                   start=True, stop=True)
            rep_sb = smallp.tile([128, 512], BF, tag="rep_sb")
            nc.scalar.copy(rep_sb[:], rep)
            nc.vector.tensor_tensor(
                akT[:, 2 * cp:2 * cp + 2, cols],
                pst.rearrange("p (a b) -> p a b", b=CT),
                rep_sb[:].rearrange("p (a b) -> p a b", b=CT), TT.mult)

    # ================= S8: kb proj + final accumulate -> out =================
    wpk = load_w("wpk", 8, C)
    for ti in range(NTC):
        cols = slice(ti * CT, (ti + 1) * CT)
        for mp in range(6):
            ps = psG(512)
            for mh in range(2):
                m = 2 * mp + mh
                half = ps[:, mh * CT:(mh + 1) * CT]
                for k in range(8):
                    MM(half, wpk[:, k, m * 128:(m + 1) * 128], akT[:, k, cols],
                       start=(k == 0), stop=(k == 7))
            sl = slice(2 * mp, 2 * mp + 2)
            pf = smallp.tile([128, 2, CT], F32, tag="istage")
            dma(pf[:], partial[:, ti, sl])
            ob = smallp.tile([128, 2, CT], F32, tag="ostage")
            nc.vector.tensor_tensor(ob[:], ps.rearrange("p (a b) -> p a b", b=CT),
                                    pf[:], TT.add)
            dma(out_ext.ap()[:, ti, sl], ob[:])

    for p in reversed(ctxs):
        p.__exit__(None, None, None)


# ------------------------------------------------------------------ entry
def kernel(**inputs):
    per_core = _prep_host(inputs)
    if "graph" not in _CACHE:
        _CACHE["graph"] = build_graph(per_core[0])
    nc = _CACHE["graph"]
    res = run_bass_kernel_spmd(nc, per_core, core_ids=list(range(NCORES)))
    outs = []
    for i in range(NCORES):
        o = np.asarray(res.results[i]["out"])          # [128, NTC, 12, CT]
        o = o.transpose(2, 0, 1, 3).reshape(C, NSC, T)  # [C, s_local, t]
        outs.append(o)
    full = np.concatenate(outs, axis=1)[:, :S, :]       # [C, 880, 16]
    out = np.ascontiguousarray(np.transpose(full, (2, 1, 0)))
    return out.reshape(1, T * S, C).astype(np.float32)
